# revision 25
# baseline (speedup 1.0000x reference)
"""BasisVQ (gumbel-softmax VQ codebook) Trainium2 kernel.

Math: with hard straight-through, one_hot == y_hard exactly in fp arithmetic
(y_soft - y_soft == 0), so the einsums are exact row gathers:
    indices[b,k]  = argmax_c(logits + gumbel)      (softmax is monotone)
    color[b,k,:]  = color_basis[indices[b,k], :]
    pos[b,k,:]    = pos_basis[indices[b,k], :]
    entropy       = -sum(avg_p * log(avg_p + 1e-8)), avg_p = mean softmax(logits)

Strategy (8 NeuronCores, data-parallel over rows = B*K; memory-bound, so the
kernel is structured to keep aggregate DMA busy ~100%):
  per core, per 128-row tile: HWDGE DMA logits/noise in (SP queue); ACT
  computes ln(u) then ln(-ln u) (one fused ACT table set for Ln+Exp); DVE
  computes z = logits + gumbel and argmax via MAX8 + FIND_INDEX_8; ACT
  computes exp(logits) with per-row accum; DVE folds softmax rows into a
  [128,1024] accumulator; GPSIMD indirect DMA gathers the fused
  [1024,2700] basis row per index from DRAM into SBUF; ACT-queue HWDGE
  DMA writes color/pos rows out. Final probs partition-reduction via two
  PE matmuls with a ones vector, a [1,1024] AllReduce across the 8 cores,
  and the entropy formula evaluated on device. Host: concat shards and
  fp32 re-check of rows whose top-2 z margin is below a small threshold
  (ACT Ln LUT ulp guard; ~1e-3 of rows).
"""
import os
import sys

sys.path.insert(0, "/opt/trn_rl_repo")

import numpy as np

B, K, C = 128, 512, 1024
CD, PD = 2250, 450
D = CD + PD
NCORES = 8
R = (B * K) // NCORES  # 8192 rows per core
P = 128
NT = R // P  # 64 tiles per core

# rows whose top-2 z margin is below this get re-checked on host in fp32
MARGIN_PATCH_THRESHOLD = 1e-3

_compiled_nc = None
last_perf = None  # filled with BassKernelResults metadata by kernel()
last_patched = 0  # rows host-re-checked on the last call (margin guard)


def _build():
    import concourse.bacc as bacc
    import concourse.bass as bass
    import concourse.tile as tile
    from concourse import mybir

    f32 = mybir.dt.float32
    Act = mybir.ActivationFunctionType
    Alu = mybir.AluOpType

    # Both Ln and Exp live in the 'natural_log_exp_and_others' ACT table set;
    # the greedy table-load pass would otherwise pick single-function sets and
    # reload the table twice per tile (128 InstLoadActFuncSet, ~171us/core).
    # Mutate the cached set-membership so only the combined set satisfies
    # Ln/Exp; set ids keep their true act_info.json indices.
    from concourse.hw_specs import get_activation_tables

    tabs = get_activation_tables("gen3")
    if "natural_log_exp_and_others" in tabs:
        for name, funcs in tabs.items():
            if name != "natural_log_exp_and_others":
                funcs.discard(Act.Ln)
                funcs.discard(Act.Exp)

    nc = bacc.Bacc("TRN2", target_bir_lowering=False, debug=False, num_devices=NCORES)
    logits_d = nc.dram_tensor("logits", [R, C], f32, kind="ExternalInput")
    noise_d = nc.dram_tensor("noise", [R, C], f32, kind="ExternalInput")
    basis_d = nc.dram_tensor("basis", [C, D], f32, kind="ExternalInput")
    color_d = nc.dram_tensor("color", [R, CD], f32, kind="ExternalOutput")
    pos_d = nc.dram_tensor("pos", [R, PD], f32, kind="ExternalOutput")
    idx_d = nc.dram_tensor("idx_stage", [P, NT], mybir.dt.int32, kind="ExternalOutput")
    mar_d = nc.dram_tensor("margin_stage", [P, NT], f32, kind="ExternalOutput")
    ent_d = nc.dram_tensor("ent", [1, 1], f32, kind="ExternalOutput")
    cc_in = nc.dram_tensor("cc_in", [1, C], f32, kind="Internal")
    cc_out = nc.dram_tensor("cc_out", [1, C], f32, kind="Internal", addr_space="Shared")

    with tile.TileContext(nc) as tc:
        with tc.tile_pool(name="io", bufs=3) as io, \
             tc.tile_pool(name="gtp", bufs=4) as gtp, \
             tc.tile_pool(name="work", bufs=2) as work, \
             tc.tile_pool(name="small", bufs=6) as small, \
             tc.tile_pool(name="persist", bufs=1) as persist, \
             tc.tile_pool(name="psum_p", bufs=2, space="PSUM") as psum_p:
            acc = persist.tile([P, C], f32)
            nc.vector.memset(acc[:], 0.0)
            stage_idx = persist.tile([P, NT], mybir.dt.int32)
            stage_mar = persist.tile([P, NT], f32)
            ones = persist.tile([P, 1], f32)
            nc.vector.memset(ones[:], 1.0)

            # process 2 row-tiles (256 rows) per iteration: DMAs move
            # 1-2.7 MiB per instruction for better bandwidth efficiency
            BT = 2
            for it in range(NT // BT):
                i0 = it * BT
                r0 = i0 * P
                nrows = BT * P
                lt = io.tile([P, BT, C], f32, tag="lt")
                nc.sync.dma_start(
                    out=lt[:],
                    in_=logits_d[r0 : r0 + nrows, :].rearrange("(t p) c -> p t c", p=P),
                )
                ut = io.tile([P, BT, C], f32, tag="ut")
                nc.sync.dma_start(
                    out=ut[:],
                    in_=noise_d[r0 : r0 + nrows, :].rearrange("(t p) c -> p t c", p=P),
                )

                # gumbel: b = ln(-ln u); z = logits - b
                lnu = work.tile([P, BT, C], f32, tag="lnu")
                nc.scalar.activation(lnu[:], ut[:], Act.Ln)
                nb = work.tile([P, BT, C], f32, tag="nb")
                nc.scalar.activation(nb[:], lnu[:], Act.Ln, scale=-1.0)
                z = work.tile([P, BT, C], f32, tag="z")
                nc.vector.tensor_sub(z[:], lt[:], nb[:])

                e = work.tile([P, BT, C], f32, tag="lnu")
                for t in range(BT):
                    i = i0 + t
                    # argmax over C via top-8 + index match
                    mx8 = small.tile([P, 8], f32, tag="mx8")
                    nc.vector.max(mx8[:], z[:, t, :])
                    idx8 = small.tile([P, 8], mybir.dt.uint32, tag="idx8")
                    nc.vector.max_index(idx8[:], mx8[:], z[:, t, :])
                    nc.vector.tensor_copy(
                        stage_idx[:, i : i + 1], idx8[:, 0:1].bitcast(mybir.dt.int32)
                    )
                    nc.vector.tensor_sub(
                        stage_mar[:, i : i + 1], mx8[:, 0:1], mx8[:, 1:2]
                    )

                    # entropy partials: probs = exp(l) / rowsum(exp(l))
                    # (no max-subtraction needed: |logits| < ~7 so exp is safe)
                    s = small.tile([P, 1], f32, tag="s")
                    nc.scalar.activation(
                        e[:, t, :], lt[:, t, :], Act.Exp, accum_out=s[:]
                    )
                    rs = small.tile([P, 1], f32, tag="rs")
                    nc.vector.reciprocal(rs[:], s[:])
                    nc.vector.scalar_tensor_tensor(
                        out=acc[:], in0=e[:, t, :], scalar=rs[:], in1=acc[:],
                        op0=Alu.mult, op1=Alu.add,
                    )

                # gather fused basis rows (one indirect DMA per 128-row tile;
                # the DGE only accepts one offset per partition)
                gt = gtp.tile([P, BT, D], f32, tag="gt")
                for t in range(BT):
                    nc.gpsimd.indirect_dma_start(
                        out=gt[:, t, :],
                        out_offset=None,
                        in_=basis_d[:, :],
                        in_offset=bass.IndirectOffsetOnAxis(
                            ap=stage_idx[:, i0 + t : i0 + t + 1], axis=0
                        ),
                    )
                nc.scalar.dma_start(
                    out=color_d[r0 : r0 + nrows, :].rearrange("(t p) d -> p t d", p=P),
                    in_=gt[:, :, 0:CD],
                )
                nc.scalar.dma_start(
                    out=pos_d[r0 : r0 + nrows, :].rearrange("(t p) d -> p t d", p=P),
                    in_=gt[:, :, CD:D],
                )

            # small outputs first, so they schedule inside the store shadow
            # rather than after the collective tail
            nc.sync.dma_start(out=idx_d[:, :], in_=stage_idx[:])
            nc.sync.dma_start(out=mar_d[:, :], in_=stage_mar[:])

            # partition-reduce acc [128,1024] -> [1,1024] via PE with ones,
            # all-reduce across the 8 cores, then entropy on device
            # (tail tiles borrow slots from the loop pools — no extra SBUF)
            pp_sb = work.tile([1, C], f32, tag="lnu")
            for h in range(2):
                ps = psum_p.tile([1, 512], f32, tag="ps")
                nc.tensor.matmul(
                    out=ps[:], lhsT=ones[:], rhs=acc[:, h * 512 : (h + 1) * 512],
                    start=True, stop=True,
                )
                nc.vector.tensor_copy(pp_sb[:, h * 512 : (h + 1) * 512], ps[:])
            nc.sync.dma_start(out=cc_in[:, :], in_=pp_sb[:])
            nc.gpsimd.collective_compute(
                "AllReduce",
                Alu.add,
                replica_groups=[list(range(NCORES))],
                ins=[cc_in[:, :]],
                outs=[cc_out[:, :]],
            )
            tot = work.tile([1, C], f32, tag="nb")
            nc.sync.dma_start(out=tot[:], in_=cc_out[:, :])
            avg = work.tile([1, C], f32, tag="z")
            nc.vector.tensor_scalar_mul(avg[:], tot[:], 1.0 / float(B * K))
            biast = small.tile([1, 1], f32, tag="s")
            nc.vector.memset(biast[:], 1e-8)
            lnt = work.tile([1, C], f32, tag="lnu")
            nc.scalar.activation(lnt[:], avg[:], Act.Ln, bias=biast[:])
            prod = work.tile([1, C], f32, tag="nb")
            nc.vector.tensor_tensor(out=prod[:], in0=avg[:], in1=lnt[:], op=Alu.mult)
            ent = small.tile([1, 1], f32, tag="rs")
            nc.vector.tensor_reduce(
                ent[:], prod[:], axis=mybir.AxisListType.X, op=Alu.add, negate=True
            )
            nc.sync.dma_start(out=ent_d[:, :], in_=ent[:])

    nc.compile()
    return nc


def kernel(**inputs):
    global _compiled_nc, last_perf
    from concourse.bass_utils import run_bass_kernel_spmd

    logits = np.ascontiguousarray(np.asarray(inputs["logits"], dtype=np.float32)).reshape(B * K, C)
    noise = np.ascontiguousarray(np.asarray(inputs["noise_u"], dtype=np.float32)).reshape(B * K, C)
    color_basis = np.asarray(inputs["color_basis"], dtype=np.float32)
    pos_basis = np.asarray(inputs["pos_basis"], dtype=np.float32)
    basis = np.ascontiguousarray(np.concatenate([color_basis, pos_basis], axis=1))

    if _compiled_nc is None:
        _compiled_nc = _build()
    nc = _compiled_nc

    in_maps = []
    for c in range(NCORES):
        sl = slice(c * R, (c + 1) * R)
        in_maps.append({"logits": logits[sl], "noise": noise[sl], "basis": basis})

    trace = os.environ.get("BASISVQ_TRACE") == "1"
    try:
        res = run_bass_kernel_spmd(
            nc, in_maps, core_ids=list(range(NCORES)), trace=trace
        )
    except (ImportError, ModuleNotFoundError):
        # BASS_TRACE=1 environments without the axon NTFF hook module crash
        # inside run_bass_kernel_spmd's trace path; retry untraced.
        os.environ["BASS_NEVER_TRACE"] = "1"
        res = run_bass_kernel_spmd(
            nc, in_maps, core_ids=list(range(NCORES)), trace=False
        )
    last_perf = res
    rs = res.results

    color = np.concatenate([r["color"] for r in rs], axis=0)
    pos = np.concatenate([r["pos"] for r in rs], axis=0)
    # stage layout: stage[p, t] = row (t*128 + p) of that core's shard
    idx = np.concatenate(
        [r["idx_stage"].T.reshape(-1) for r in rs]
    ).astype(np.int32)
    margins = np.concatenate([r["margin_stage"].T.reshape(-1) for r in rs])

    # fp32 host re-check of numerically ambiguous rows (ACT Ln LUT has ~3e-6
    # abs error on ln(-ln u); rows with a tiny top-2 margin could flip argmax
    # relative to an exactly-rounded fp32 reference)
    suspect = np.nonzero(margins < MARGIN_PATCH_THRESHOLD)[0]
    global last_patched
    last_patched = int(suspect.size)
    if suspect.size:
        lg = logits[suspect]
        u = noise[suspect]
        zz = lg + (-np.log(-np.log(u)))
        new_idx = np.argmax(zz, axis=1).astype(np.int32)
        idx[suspect] = new_idx
        color[suspect] = color_basis[new_idx]
        pos[suspect] = pos_basis[new_idx]

    # entropy was all-reduced + computed on device; identical on every core
    entropy = np.float32(rs[0]["ent"][0, 0])

    return (
        color.reshape(B, K, CD),
        pos.reshape(B, K, PD),
        idx.reshape(B, K),
        entropy,
    )


# revision 26
# speedup vs baseline: 1.0009x; 1.0009x over previous
"""BasisVQ (gumbel-softmax VQ codebook) Trainium2 kernel.

Math: with hard straight-through, one_hot == y_hard exactly in fp arithmetic
(y_soft - y_soft == 0), so the einsums are exact row gathers:
    indices[b,k]  = argmax_c(logits + gumbel)      (softmax is monotone)
    color[b,k,:]  = color_basis[indices[b,k], :]
    pos[b,k,:]    = pos_basis[indices[b,k], :]
    entropy       = -sum(avg_p * log(avg_p + 1e-8)), avg_p = mean softmax(logits)

Strategy (8 NeuronCores, data-parallel over rows = B*K; memory-bound, so the
kernel is structured to keep aggregate DMA busy ~100%):
  per core, per 128-row tile: HWDGE DMA logits/noise in (SP queue); ACT
  computes ln(u) then ln(-ln u) (one fused ACT table set for Ln+Exp); DVE
  computes z = logits + gumbel and argmax via MAX8 + FIND_INDEX_8; ACT
  computes exp(logits) with per-row accum; DVE folds softmax rows into a
  [128,1024] accumulator; GPSIMD indirect DMA gathers the fused
  [1024,2700] basis row per index from DRAM into SBUF; ACT-queue HWDGE
  DMA writes color/pos rows out. Final probs partition-reduction via two
  PE matmuls with a ones vector, a [1,1024] AllReduce across the 8 cores,
  and the entropy formula evaluated on device. Host: concat shards and
  fp32 re-check of rows whose top-2 z margin is below a small threshold
  (ACT Ln LUT ulp guard; ~1e-3 of rows).
"""
import os
import sys

sys.path.insert(0, "/opt/trn_rl_repo")

import numpy as np

B, K, C = 128, 512, 1024
CD, PD = 2250, 450
D = CD + PD
NCORES = 8
R = (B * K) // NCORES  # 8192 rows per core
P = 128
NT = R // P  # 64 tiles per core

# rows whose top-2 z margin is below this get re-checked on host in fp32
MARGIN_PATCH_THRESHOLD = 1e-3

_compiled_nc = None
last_perf = None  # filled with BassKernelResults metadata by kernel()
last_patched = 0  # rows host-re-checked on the last call (margin guard)


def _build():
    import concourse.bacc as bacc
    import concourse.bass as bass
    import concourse.tile as tile
    from concourse import mybir

    f32 = mybir.dt.float32
    Act = mybir.ActivationFunctionType
    Alu = mybir.AluOpType

    # Both Ln and Exp live in the 'natural_log_exp_and_others' ACT table set;
    # the greedy table-load pass would otherwise pick single-function sets and
    # reload the table twice per tile (128 InstLoadActFuncSet, ~171us/core).
    # Mutate the cached set-membership so only the combined set satisfies
    # Ln/Exp; set ids keep their true act_info.json indices.
    from concourse.hw_specs import get_activation_tables

    tabs = get_activation_tables("gen3")
    if "natural_log_exp_and_others" in tabs:
        for name, funcs in tabs.items():
            if name != "natural_log_exp_and_others":
                funcs.discard(Act.Ln)
                funcs.discard(Act.Exp)

    nc = bacc.Bacc("TRN2", target_bir_lowering=False, debug=False, num_devices=NCORES)
    logits_d = nc.dram_tensor("logits", [R, C], f32, kind="ExternalInput")
    noise_d = nc.dram_tensor("noise", [R, C], f32, kind="ExternalInput")
    basis_d = nc.dram_tensor("basis", [C, D], f32, kind="ExternalInput")
    color_d = nc.dram_tensor("color", [R, CD], f32, kind="ExternalOutput")
    pos_d = nc.dram_tensor("pos", [R, PD], f32, kind="ExternalOutput")
    idx_d = nc.dram_tensor("idx_stage", [P, NT], mybir.dt.int32, kind="ExternalOutput")
    mar_d = nc.dram_tensor("margin_stage", [P, NT], f32, kind="ExternalOutput")
    ent_d = nc.dram_tensor("ent", [1, 1], f32, kind="ExternalOutput")
    cc_in = nc.dram_tensor("cc_in", [1, C], f32, kind="Internal")
    cc_out = nc.dram_tensor("cc_out", [1, C], f32, kind="Internal", addr_space="Shared")

    with tile.TileContext(nc) as tc:
        with tc.tile_pool(name="io", bufs=3) as io, \
             tc.tile_pool(name="gtp", bufs=4) as gtp, \
             tc.tile_pool(name="work", bufs=2) as work, \
             tc.tile_pool(name="small", bufs=6) as small, \
             tc.tile_pool(name="persist", bufs=1) as persist, \
             tc.tile_pool(name="psum_p", bufs=2, space="PSUM") as psum_p:
            acc = persist.tile([P, C], f32)
            nc.vector.memset(acc[:], 0.0)
            stage_idx = persist.tile([P, NT], mybir.dt.int32)
            stage_mar = persist.tile([P, NT], f32)
            ones = persist.tile([P, 1], f32)
            nc.vector.memset(ones[:], 1.0)

            # process 2 row-tiles (256 rows) per iteration: DMAs move
            # 1-2.7 MiB per instruction for better bandwidth efficiency
            BT = 2
            for it in range(NT // BT):
                i0 = it * BT
                r0 = i0 * P
                nrows = BT * P
                lt = io.tile([P, BT, C], f32, tag="lt")
                nc.sync.dma_start(
                    out=lt[:],
                    in_=logits_d[r0 : r0 + nrows, :].rearrange("(t p) c -> p t c", p=P),
                )
                ut = io.tile([P, BT, C], f32, tag="ut")
                nc.sync.dma_start(
                    out=ut[:],
                    in_=noise_d[r0 : r0 + nrows, :].rearrange("(t p) c -> p t c", p=P),
                )

                # gumbel: b = ln(-ln u); z = logits - b
                lnu = work.tile([P, BT, C], f32, tag="lnu")
                nc.scalar.activation(lnu[:], ut[:], Act.Ln)
                nb = work.tile([P, BT, C], f32, tag="nb")
                nc.scalar.activation(nb[:], lnu[:], Act.Ln, scale=-1.0)
                z = work.tile([P, BT, C], f32, tag="z")
                nc.vector.tensor_sub(z[:], lt[:], nb[:])

                e = work.tile([P, BT, C], f32, tag="lnu")
                for t in range(BT):
                    i = i0 + t
                    # argmax over C via top-8 + index match
                    mx8 = small.tile([P, 8], f32, tag="mx8")
                    nc.vector.max(mx8[:], z[:, t, :])
                    idx8 = small.tile([P, 8], mybir.dt.uint32, tag="idx8")
                    nc.vector.max_index(idx8[:], mx8[:], z[:, t, :])
                    nc.vector.tensor_copy(
                        stage_idx[:, i : i + 1], idx8[:, 0:1].bitcast(mybir.dt.int32)
                    )
                    nc.vector.tensor_sub(
                        stage_mar[:, i : i + 1], mx8[:, 0:1], mx8[:, 1:2]
                    )

                    # entropy partials: probs = exp(l) / rowsum(exp(l))
                    # (no max-subtraction needed: |logits| < ~7 so exp is safe)
                    s = small.tile([P, 1], f32, tag="s")
                    nc.scalar.activation(
                        e[:, t, :], lt[:, t, :], Act.Exp, accum_out=s[:]
                    )
                    rs = small.tile([P, 1], f32, tag="rs")
                    nc.vector.reciprocal(rs[:], s[:])
                    nc.vector.scalar_tensor_tensor(
                        out=acc[:], in0=e[:, t, :], scalar=rs[:], in1=acc[:],
                        op0=Alu.mult, op1=Alu.add,
                    )

                # gather fused basis rows (one indirect DMA per 128-row tile;
                # the DGE only accepts one offset per partition)
                gt = gtp.tile([P, BT, D], f32, tag="gt")
                for t in range(BT):
                    nc.gpsimd.indirect_dma_start(
                        out=gt[:, t, :],
                        out_offset=None,
                        in_=basis_d[:, :],
                        in_offset=bass.IndirectOffsetOnAxis(
                            ap=stage_idx[:, i0 + t : i0 + t + 1], axis=0
                        ),
                    )
                nc.scalar.dma_start(
                    out=color_d[r0 : r0 + nrows, :].rearrange("(t p) d -> p t d", p=P),
                    in_=gt[:, :, 0:CD],
                )
                nc.scalar.dma_start(
                    out=pos_d[r0 : r0 + nrows, :].rearrange("(t p) d -> p t d", p=P),
                    in_=gt[:, :, CD:D],
                )

            # small outputs first, so they schedule inside the store shadow
            # rather than after the collective tail
            nc.sync.dma_start(out=idx_d[:, :], in_=stage_idx[:])
            nc.sync.dma_start(out=mar_d[:, :], in_=stage_mar[:])

            # partition-reduce acc [128,1024] -> [1,1024] via PE with ones,
            # all-reduce across the 8 cores, then entropy on device
            # (tail tiles borrow slots from the loop pools — no extra SBUF)
            pp_sb = work.tile([1, C], f32, tag="lnu")
            for h in range(2):
                ps = psum_p.tile([1, 512], f32, tag="ps")
                nc.tensor.matmul(
                    out=ps[:], lhsT=ones[:], rhs=acc[:, h * 512 : (h + 1) * 512],
                    start=True, stop=True,
                )
                nc.vector.tensor_copy(pp_sb[:, h * 512 : (h + 1) * 512], ps[:])
            nc.sync.dma_start(out=cc_in[:, :], in_=pp_sb[:])
            nc.gpsimd.collective_compute(
                "AllReduce",
                Alu.add,
                replica_groups=[list(range(NCORES))],
                ins=[cc_in[:, :]],
                outs=[cc_out[:, :]],
            )
            tot = work.tile([1, C], f32, tag="nb")
            nc.sync.dma_start(out=tot[:], in_=cc_out[:, :])
            # entropy = -sum(avg*ln(avg+1e-8)), avg = tot/65536: fold the mean
            # scale into Ln's input scale, and the avg factor + sign into the
            # final [1,1] scalar: -c*sum(tot*ln(c*tot + 1e-8))
            biast = small.tile([1, 1], f32, tag="s")
            nc.vector.memset(biast[:], 1e-8)
            lnt = work.tile([1, C], f32, tag="lnu")
            nc.scalar.activation(
                lnt[:], tot[:], Act.Ln, bias=biast[:], scale=1.0 / float(B * K)
            )
            prod = work.tile([1, C], f32, tag="z")
            nc.vector.tensor_tensor(out=prod[:], in0=tot[:], in1=lnt[:], op=Alu.mult)
            ent0 = small.tile([1, 1], f32, tag="rs")
            nc.vector.tensor_reduce(
                ent0[:], prod[:], axis=mybir.AxisListType.X, op=Alu.add
            )
            ent = small.tile([1, 1], f32, tag="mx8")
            nc.vector.tensor_scalar_mul(ent[:], ent0[:], -1.0 / float(B * K))
            nc.sync.dma_start(out=ent_d[:, :], in_=ent[:])

    nc.compile()
    return nc


def kernel(**inputs):
    global _compiled_nc, last_perf
    from concourse.bass_utils import run_bass_kernel_spmd

    logits = np.ascontiguousarray(np.asarray(inputs["logits"], dtype=np.float32)).reshape(B * K, C)
    noise = np.ascontiguousarray(np.asarray(inputs["noise_u"], dtype=np.float32)).reshape(B * K, C)
    color_basis = np.asarray(inputs["color_basis"], dtype=np.float32)
    pos_basis = np.asarray(inputs["pos_basis"], dtype=np.float32)
    basis = np.ascontiguousarray(np.concatenate([color_basis, pos_basis], axis=1))

    if _compiled_nc is None:
        _compiled_nc = _build()
    nc = _compiled_nc

    in_maps = []
    for c in range(NCORES):
        sl = slice(c * R, (c + 1) * R)
        in_maps.append({"logits": logits[sl], "noise": noise[sl], "basis": basis})

    trace = os.environ.get("BASISVQ_TRACE") == "1"
    try:
        res = run_bass_kernel_spmd(
            nc, in_maps, core_ids=list(range(NCORES)), trace=trace
        )
    except (ImportError, ModuleNotFoundError):
        # BASS_TRACE=1 environments without the axon NTFF hook module crash
        # inside run_bass_kernel_spmd's trace path; retry untraced.
        os.environ["BASS_NEVER_TRACE"] = "1"
        res = run_bass_kernel_spmd(
            nc, in_maps, core_ids=list(range(NCORES)), trace=False
        )
    last_perf = res
    rs = res.results

    color = np.concatenate([r["color"] for r in rs], axis=0)
    pos = np.concatenate([r["pos"] for r in rs], axis=0)
    # stage layout: stage[p, t] = row (t*128 + p) of that core's shard
    idx = np.concatenate(
        [r["idx_stage"].T.reshape(-1) for r in rs]
    ).astype(np.int32)
    margins = np.concatenate([r["margin_stage"].T.reshape(-1) for r in rs])

    # fp32 host re-check of numerically ambiguous rows (ACT Ln LUT has ~3e-6
    # abs error on ln(-ln u); rows with a tiny top-2 margin could flip argmax
    # relative to an exactly-rounded fp32 reference)
    suspect = np.nonzero(margins < MARGIN_PATCH_THRESHOLD)[0]
    global last_patched
    last_patched = int(suspect.size)
    if suspect.size:
        lg = logits[suspect]
        u = noise[suspect]
        zz = lg + (-np.log(-np.log(u)))
        new_idx = np.argmax(zz, axis=1).astype(np.int32)
        idx[suspect] = new_idx
        color[suspect] = color_basis[new_idx]
        pos[suspect] = pos_basis[new_idx]

    # entropy was all-reduced + computed on device; identical on every core
    entropy = np.float32(rs[0]["ent"][0, 0])

    return (
        color.reshape(B, K, CD),
        pos.reshape(B, K, PD),
        idx.reshape(B, K),
        entropy,
    )
